# revision 34
# baseline (speedup 1.0000x reference)
"""Multi-head attention (B=2, S=2048, D=1024, H=16, hd=64) on 8 TRN2 cores.

Sharding: tensor-parallel on heads x data-parallel on batch. Core c handles
batch b=c//4 and heads [4*(c%4), 4*(c%4)+4). Each core computes the fused QKV
projection for its head slice, attention for its 4 heads, and a partial
out-projection (out_w column slice); the host sums the 4 partials per batch
and adds out_b.

On-device layout (all matmul inputs fp32r -> full-rate PE):
- qk^T [e, s]: scores^T tiles need dd on partitions; produced directly by
  lhsT=W^T, rhs=x^T. q weights+bias pre-scaled by 1/sqrt(hd) on host.
- v natural [s, dd] (+ ones column per head): pv matmul lhsT=v gives
  vals^T [dd, i] AND the softmax row sums in psum row 64.
- scores^T via K=64 row-packed head pairs (partitions 0-63 / 64-127).
- exp on ACT over [128, 2048] psum regions (amortizes ACT overhead).
- normalize vals^T with reciprocal row sums broadcast via sbuf->sbuf DMA.
"""

import ml_dtypes
import numpy as np

import concourse.bass as bass
import concourse.mybir as mybir
import concourse.tile as tile
from concourse.bass_utils import run_bass_kernel_spmd
from concourse.vector_clock import ScopedClock

B = 2
S = 2048
D = 1024
NH = 16
HD = 64
NCORES = 8
HPC = 4            # heads per core
F32 = mybir.dt.float32
F32R = mybir.dt.float32r
BF16 = mybir.dt.bfloat16

SCALE = 1.0 / np.sqrt(HD)
KT = D // 128      # 8 contraction tiles for the projections
NI = S // 512      # 4 i-chunks (query)
NJ = S // 128      # 16 j-tiles (key)
VW = HD + 2        # 66: v columns + ones column + pad (4B-aligned bf16 slices)


# ---------------------------------------------------------------------------
# Walrus workaround: this toolchain rejects instructions carrying more than
# one sem wait. Split excess waits onto injected same-engine NoOps placed
# directly before the instruction (same-engine program order preserves
# semantics). DMA completion updates are left untouched.
# ---------------------------------------------------------------------------

def _patched_drain_and_barrier(self, tick_clock, wait_clock):
    nc = self.nc
    collector = nc.sync.nop()
    wait_clock.add_sem_waits(
        collector.ins, ScopedClock({None: tick_clock.global_clock})
    )
    si = collector.ins.sync_info
    if si is not None:
        waits = list(si.on_wait or [])
        if len(waits) > 1:
            collector.ins.sync_info = mybir.SyncInfo(
                on_wait=[waits[0]], on_update=list(si.on_update or [])
            )
            for w in waits[1:]:
                n2 = nc.sync.nop()
                n2.ins.sync_info = mybir.SyncInfo(on_wait=[w], on_update=[])
    nc.sync.drain()
    nc.all_engine_barrier()
    popped = nc._tile_sem_poison_stack.pop()
    assert popped is self._sem_poison
    nc.clear_and_free_semaphores(list(self.sems.allocated().values()))
    nc.all_engine_barrier()


def _split_excess_waits(nc, limit=1):
    import bass_rust

    n_split = 0
    for f in nc.m.functions:
        for bb in f.blocks:
            out = []
            for inst in bb.instructions:
                si = inst.sync_info
                waits = list(si.on_wait) if si is not None and si.on_wait else []
                lim = limit
                if len(waits) > lim:
                    for w in waits[:-lim]:
                        nop = bass_rust.InstNoOp(
                            name=f"{inst.name}-waitsplit-{n_split}",
                            ins=[], outs=[], engine=inst.engine,
                        )
                        nop.sync_info = mybir.SyncInfo(on_wait=[w], on_update=[])
                        out.append(nop)
                        n_split += 1
                    inst.sync_info = mybir.SyncInfo(
                        on_wait=waits[-lim:],
                        on_update=list(si.on_update) if si.on_update else [],
                    )
                out.append(inst)
            bb.instructions[:] = out
    return n_split


# ---------------------------------------------------------------------------
# Program build (identical SPMD program on all 8 cores; shards differ in data)
# ---------------------------------------------------------------------------

def _build_program(reps=1, phase="full"):
    """reps>1 repeats the whole computation (idempotently) inside one NEFF —
    used by the benchmark to separate HW exec time from dispatch overhead.
    phase: "proj" | "attn" | "full" — truncated variants for HW bisection."""
    tile.TileContext._drain_and_barrier = _patched_drain_and_barrier

    nc = bass.Bass("TRN2", target_bir_lowering=False, debug=False,
                   num_devices=NCORES)

    xT = nc.dram_tensor("xT", [D, S], BF16, kind="ExternalInput").ap()
    wqk = nc.dram_tensor("wqk", [D, 2 * HPC * HD], BF16, kind="ExternalInput").ap()
    bqk = nc.dram_tensor("bqk", [128, 2 * HPC * HD // 128], F32, kind="ExternalInput").ap()
    wv = nc.dram_tensor("wv", [D, HPC * HD], BF16, kind="ExternalInput").ap()
    bvr = nc.dram_tensor("bvr", [128, HPC * HD], F32, kind="ExternalInput").ap()
    ot = nc.dram_tensor("ot", [2 * 128, D], BF16, kind="ExternalInput").ap()
    ones_in = nc.dram_tensor("ones_in", [128, 64], F32R, kind="ExternalInput").ap()
    out_p = nc.dram_tensor("out_p", [S, D], F32, kind="ExternalOutput").ap()

    NQK = 2 * HPC * HD // 128     # 4 e-tiles of qk^T

    with tile.TileContext(nc) as tc:
        with (
            nc.allow_low_precision(reason="fp32r matmul input rounding"),
            tc.tile_pool(name="weights", bufs=1) as wpool,
            tc.tile_pool(name="acts", bufs=1) as apool,
            tc.tile_pool(name="work", bufs=1) as workpool,
        ):
            # ---------------- loads ----------------
            xT_sb = [wpool.tile([128, S], BF16, tag=f"xT{k}", name=f"xT{k}")
                     for k in range(KT)]
            wqk_sb = [wpool.tile([128, 2 * HPC * HD], BF16, tag=f"wqk{k}",
                                 name=f"wqk{k}") for k in range(KT)]
            wv_sb = [wpool.tile([128, HPC * HD], BF16, tag=f"wv{k}",
                                name=f"wv{k}") for k in range(KT)]
            bqk_sb = wpool.tile([128, NQK], F32, tag="bqk")
            bvr_sb = wpool.tile([128, HPC * HD], F32, tag="bvr")
            ones_sb = wpool.tile([128, 64], F32R, tag="ones")
            ot_sb = [wpool.tile([128, D], BF16, tag=f"ot{k}", name=f"ot{k}")
                     for k in range(2)]
            nc.sync.dma_start(out=bqk_sb[:], in_=bqk[:])
            nc.sync.dma_start(out=bvr_sb[:], in_=bvr[:])
            nc.sync.dma_start(out=ones_sb[:], in_=ones_in[:])
            for k in range(KT):
                nc.sync.dma_start(out=wqk_sb[k][:], in_=wqk[k * 128:(k + 1) * 128, :])
                nc.sync.dma_start(out=xT_sb[k][:], in_=xT[k * 128:(k + 1) * 128, :])
                nc.sync.dma_start(out=wv_sb[k][:], in_=wv[k * 128:(k + 1) * 128, :])
            for k in range(2):
                nc.sync.dma_start(out=ot_sb[k][:], in_=ot[k * 128:(k + 1) * 128, :])

            # persistent activation buffers
            qk_sb = [apool.tile([128, S], F32R, tag=f"qk{t}", name=f"qk{t}")
                     for t in range(NQK)]
            v_sb = [apool.tile([128, HPC * VW], BF16, tag=f"v{j}", name=f"v{j}")
                    for j in range(NJ)]
            valsT = [apool.tile([128, S], BF16, tag=f"vals{p}", name=f"vals{p}")
                     for p in range(2)]
            # constant ones columns of v (softmax row-sum trick), written once
            for j in range(NJ):
                v_dst = v_sb[j][:, 0:HPC * VW].rearrange(
                    "p (h e) -> p h e", h=HPC, e=VW)
                nc.vector.memset(v_dst[:, :, HD:VW], 1.0)

            # ---------------- single psum pool: proj/outproj share tag "pp"
            # (2 banks) + wide (4) + pv (2) = 8 banks, no phase barrier so
            # attention interleaves with late projection tiles.
            with (
                tc.tile_pool(name="ps", bufs=1, space="PSUM") as psp,
                tc.tile_pool(name="attn_sb", bufs=3) as ab,
                tc.tile_pool(name="norm_sb", bufs=2) as nb,
                tc.tile_pool(name="out_sb", bufs=3) as ob,
                tc.tile_pool(name="dram_sc", bufs=4, space="DRAM") as dsc,
            ):
                def emit_qk_tile(t, i, borrow_wide=False):
                    # one qk^T e-tile [e, s-chunk]; bias added on eviction.
                    # Early tiles may borrow the (still idle) "wide" psum
                    # slots so more accumulations overlap the input DMA.
                    if borrow_wide:
                        qs = psp.tile([128, 512], F32, tag="wide", bufs=2,
                                      name="qs")
                    else:
                        qs = psp.tile([128, 512], F32, tag="pp", bufs=2,
                                      name="qs")
                    for k in range(KT):
                        nc.tensor.matmul(
                            qs[:],
                            wqk_sb[k][:, t * 128:(t + 1) * 128],
                            xT_sb[k][:, i * 512:(i + 1) * 512],
                            start=(k == 0), stop=(k == KT - 1),
                        )
                    nc.vector.tensor_scalar_add(
                        qk_sb[t][:, i * 512:(i + 1) * 512], qs[:],
                        bqk_sb[:, t:t + 1],
                    )

                def emit_qk_proj(ts_pair, borrow_wide=False):
                    n_emitted = 0
                    for i in range(NI):
                        for t in ts_pair:
                            emit_qk_tile(t, i, borrow_wide and n_emitted < 2)
                            n_emitted += 1

                def emit_v_tile(j):
                    # v natural [s, dd] + bias (ones columns written once)
                    vs = psp.tile([128, HPC * HD], F32, tag="pp", bufs=2, name="vs")
                    for k in range(KT):
                        nc.tensor.matmul(
                            vs[:],
                            xT_sb[k][:, j * 128:(j + 1) * 128],
                            wv_sb[k][:],
                            start=(k == 0), stop=(k == KT - 1),
                        )
                    v_dst = v_sb[j][:, 0:HPC * VW].rearrange(
                        "p (h e) -> p h e", h=HPC, e=VW)
                    nc.vector.tensor_tensor(
                        v_dst[:, :, 0:HD],
                        vs[:].rearrange("p (h e) -> p h e", h=HPC, e=HD),
                        bvr_sb[:].rearrange("p (h e) -> p h e", h=HPC, e=HD),
                        op=mybir.AluOpType.add,
                    )

                def emit_v_proj():
                    for j in range(NJ):
                        emit_v_tile(j)

                def emit_attn(p, i, at_start=(), steps=None, no_pv=False):
                    # Software-pipelined: scores/exp for j are emitted one
                    # iteration ahead of the pv matmuls for j-1, so the PE
                    # (in-order) issues the next scores pair instead of
                    # blocking on exp(j) before pv(j). `at_start`/`steps`
                    # inject deferred work (normalize / out-proj of the
                    # previous chunk) into this chunk's pipeline.
                    steps = steps or {}
                    isl = slice(i * 512, (i + 1) * 512)
                    for fn in at_start:
                        fn()
                    if no_pv:
                        pvA = pvB = None
                    else:
                        pvA = psp.tile([128, 512], F32, tag="pv", bufs=2, name="pvA")
                        pvB = psp.tile([128, 512], F32, tag="pv", bufs=2, name="pvB")
                    hA, hB = 2 * p, 2 * p + 1
                    es_q = {}
                    for j in range(NJ + 1):
                        if j < NJ:
                            jsl = slice(j * 128, (j + 1) * 128)
                            wide = psp.tile([128, 1024], F32, tag="wide",
                                            bufs=2, name="wide")
                            # scores^T: row-packed pair (head A rows 0:64,
                            # head B rows 64:128 of the PE array)
                            nc.tensor.matmul(
                                wide[:, 0:512],
                                qk_sb[2 + p][0:64, jsl],
                                qk_sb[p][0:64, isl], start=True, stop=True)
                            nc.tensor.matmul(
                                wide[:, 512:1024],
                                qk_sb[2 + p][64:128, jsl],
                                qk_sb[p][64:128, isl], start=True, stop=True)
                            es = ab.tile([128, 1024], BF16, tag="es", bufs=6,
                                         name="es")
                            nc.scalar.activation(
                                es[:], wide[:], mybir.ActivationFunctionType.Exp)
                            es_q[j] = es
                        if j >= 1 and not no_pv:
                            es = es_q.pop(j - 1)
                            nc.tensor.matmul(
                                pvA[0:VW, :],
                                v_sb[j - 1][:, hA * VW:(hA + 1) * VW],
                                es[:, 0:512],
                                start=(j - 1 == 0), stop=(j - 1 == NJ - 1))
                            nc.tensor.matmul(
                                pvB[0:VW, :],
                                v_sb[j - 1][:, hB * VW:(hB + 1) * VW],
                                es[:, 512:1024],
                                start=(j - 1 == 0), stop=(j - 1 == NJ - 1))
                        for fn in steps.get(j, ()):
                            fn()
                    return pvA, pvB

                def norm_evict(pvA, pvB):
                    # evict pv psum banks asap: vals rows to raw tiles; the
                    # two sum rows packed at partitions 0/32 of one tile so a
                    # single DVE reciprocal covers both heads (its cost is
                    # free-size only; partitions run in parallel lanes)
                    raws = []
                    sums = nb.tile([33, 512], F32, tag="sums", bufs=2,
                                   name="sums")
                    for hh, pv in enumerate((pvA, pvB)):
                        raw = nb.tile([HD, 512], F32, tag="raw", bufs=3,
                                      name="raw")
                        nc.vector.tensor_copy(raw[:], pv[0:HD, :])
                        nc.vector.tensor_copy(sums[32 * hh:32 * hh + 1, :],
                                              pv[HD:HD + 1, :])
                        raws.append(raw)
                    return raws, sums

                def norm_recip(sums):
                    # one iterative DVE reciprocal for both heads (strided
                    # partition AP touches only rows 0 and 32); keeps the
                    # norm off the saturated ACT queue mid-body
                    recip = nb.tile([33, 512], F32R, tag="recip", bufs=2,
                                    name="recip")
                    nc.vector.reciprocal(recip[:], sums[:])
                    return recip

                def norm_recip_act(sums):
                    # tail variant: 1/Z = exp(-ln Z) on the (idle-at-tail)
                    # ACT engine; ln/exp share an act table so no reloads
                    recips = []
                    for hh in range(2):
                        lnz = nb.tile([1, 512], F32, tag="lnz", bufs=2,
                                      name="lnz")
                        nc.scalar.activation(
                            lnz[:], sums[32 * hh:32 * hh + 1, :],
                            mybir.ActivationFunctionType.Ln)
                        rc = nb.tile([1, 512], F32R, tag="recip2", bufs=2,
                                     name="recip2")
                        nc.scalar.activation(
                            rc[:], lnz[:],
                            mybir.ActivationFunctionType.Exp, scale=-1.0)
                        recips.append(rc)
                    return recips

                def norm_bcmm(recip):
                    # broadcast recip rows across 64 partitions via a DRAM
                    # round-trip (stride-0 read DMA); entirely off the PE,
                    # and the stt is scheduled steps later so the DMA
                    # latency never blocks the in-order DVE queue
                    bcs = []
                    for hh in range(2):
                        rd = dsc.tile([1, 512], F32R, tag="rd", name="rd")
                        nc.sync.dma_start(out=rd[:],
                                          in_=recip[32 * hh:32 * hh + 1, :])
                        bc = nb.tile([64, 512], F32R, tag="bc", bufs=3,
                                     name="bc")
                        nc.sync.dma_start(out=bc[:],
                                          in_=rd[:].to_broadcast((64, 512)))
                        bcs.append(bc)
                    return bcs

                def norm_stt(p, i, raws, bcs):
                    isl = slice(i * 512, (i + 1) * 512)
                    for hh, (raw, bc) in enumerate(zip(raws, bcs)):
                        nc.vector.scalar_tensor_tensor(
                            valsT[p][hh * 64:(hh + 1) * 64, isl],
                            raw[:], 1.0, bc[:],
                            op0=mybir.AluOpType.mult,
                            op1=mybir.AluOpType.mult,
                        )

                def emit_outproj(i, si_range=range(4)):
                    # out projection for this i-chunk
                    for si in si_range:
                        s0 = i * 512 + si * 128
                        for e in range(2):
                            op = psp.tile([128, 512], F32, tag="pp", bufs=2, name="op")
                            for k in range(2):
                                nc.tensor.matmul(
                                    op[:],
                                    valsT[k][:, s0:s0 + 128],
                                    ot_sb[k][:, e * 512:(e + 1) * 512],
                                    start=(k == 0), stop=(k == 1),
                                )
                            osb = ob.tile([128, 512], F32, tag="osb", name="osb")
                            nc.vector.tensor_copy(osb[:], op[:])
                            nc.sync.dma_start(
                                out=out_p[s0:s0 + 128, e * 512:(e + 1) * 512],
                                in_=osb[:],
                            )

                # Projections are emitted first (dep-tracking needs
                # program order = data order), but attention + out-proj get a
                # low priority band so the scheduler treats projection work
                # as filler once attention tiles become data-ready.
                attn_base = [1]

                def prio():
                    return tc.high_priority(offset=tc.cur_priority - attn_base[0])

                def emit_boot_wave():
                    # k-major first wave over all 8 psum banks: pair-0's
                    # eight qk e-tiles (t 0,2 x i 0..3) accumulate one
                    # k-chunk at a time, so the PE starts as soon as
                    # (wqk[0], xT[0]) lands instead of stalling each tile's
                    # full k-loop on the slowest DMA chunk.
                    pps = [psp.tile([128, 512], F32, tag="pp", bufs=2,
                                    name="bootpp") for _ in range(2)]
                    wides = [psp.tile([128, 1024], F32, tag="wide", bufs=2,
                                      name="bootw") for _ in range(2)]
                    pvs = [psp.tile([128, 512], F32, tag="pv", bufs=2,
                                    name="bootpv") for _ in range(2)]
                    aps = [pps[0][:], pps[1][:],
                           wides[0][:, 0:512], wides[0][:, 512:1024],
                           wides[1][:, 0:512], wides[1][:, 512:1024],
                           pvs[0][:], pvs[1][:]]
                    order = [(0, 0), (2, 0), (0, 1), (2, 1),
                             (0, 2), (2, 2), (0, 3), (2, 3)]
                    for k in range(KT):
                        for s, (t, i) in enumerate(order):
                            nc.tensor.matmul(
                                aps[s],
                                wqk_sb[k][:, t * 128:(t + 1) * 128],
                                xT_sb[k][:, i * 512:(i + 1) * 512],
                                start=(k == 0), stop=(k == KT - 1),
                            )
                    for s, (t, i) in enumerate(order):
                        nc.vector.tensor_scalar_add(
                            qk_sb[t][:, i * 512:(i + 1) * 512], aps[s],
                            bqk_sb[:, t:t + 1],
                        )

                def emit_body():
                    # boot wave gives pair-0 q/k immediately; v tiles follow
                    # (attention chunk (0, 0) consumes them j-progressively),
                    # pair-1 qk tiles last (their attention is much later).
                    emit_boot_wave()
                    for j in range(NJ):
                        emit_v_tile(j)
                    emit_qk_proj((1, 3))
                    if phase == "proj":
                        return
                    if phase == "attn_nopv":
                        for p in range(2):
                            for i in range(NI):
                                with prio():
                                    emit_attn(p, i, no_pv=True)
                                    attn_base[0] = tc.cur_priority
                        return
                    chunks = [(p, i) for p in range(2) for i in range(NI)]
                    pend = None        # previous chunk awaiting normalize
                    pend_out = None    # i-chunk awaiting out-projection
                    for (p, i) in chunks:
                        at_start = []
                        steps = {}
                        if pend is not None:
                            pp_, ii_, pvA_, pvB_ = pend
                            box = []
                            sbox = []
                            rbox = []
                            bcbox = []
                            at_start.append(
                                lambda a=pvA_, b=pvB_, bx=box, sb=sbox:
                                    (lambda rs: (bx.extend(rs[0]),
                                                 sb.append(rs[1])))(
                                        norm_evict(a, b)))
                            if phase in ("full", "noout"):
                                steps.setdefault(1, []).append(
                                    lambda sb=sbox, rb=rbox: rb.append(
                                        norm_recip(sb[0])))
                                steps.setdefault(5, []).append(
                                    lambda rb=rbox, bb=bcbox: bb.extend(
                                        norm_bcmm(rb[0])))
                                steps.setdefault(8, []).append(
                                    lambda p2=pp_, i2=ii_, bx=box, bb=bcbox:
                                        norm_stt(p2, i2, bx, bb))
                            if pp_ == 1 and phase == "full":
                                # out-projection for ii_ right after its stt
                                for k, si in enumerate((10, 12, 14, 15)):
                                    steps.setdefault(si, []).append(
                                        lambda i2=ii_, k2=k: emit_outproj(
                                            i2, range(k2, k2 + 1)))
                                pend_out = None
                        with prio():
                            pvA, pvB = emit_attn(p, i, at_start, steps)
                            attn_base[0] = tc.cur_priority
                        pend = (p, i, pvA, pvB)
                    # tail: last chunk's normalize + final out-projections,
                    # pipelined at si (128-query) granularity so the first
                    # out-proj matmuls start ~3 stt-slices earlier
                    with prio():
                        pp_, ii_, pvA_, pvB_ = pend
                        raws, sums = norm_evict(pvA_, pvB_)
                        if phase in ("full", "noout"):
                            # broadcast via K=1 matmul — the PE is idle in
                            # the tail and psum banks are free, so this
                            # replaces the ~4.5us DMA round-trip latency
                            # with a ~0.25us matmul
                            bcs = []
                            for recip in norm_recip_act(sums):
                                bcps = psp.tile([128, 512], F32, tag="pv",
                                                bufs=2, name="bcps")
                                nc.tensor.matmul(
                                    bcps[0:64, :], ones_sb[0:1, 0:64],
                                    recip[:], start=True, stop=True)
                                bcs.append(bcps)
                            for si in range(4):
                                ssl = slice(si * 128, (si + 1) * 128)
                                csl = slice(ii_ * 512 + si * 128,
                                            ii_ * 512 + (si + 1) * 128)
                                for hh, (raw, bc) in enumerate(zip(raws, bcs)):
                                    nc.vector.scalar_tensor_tensor(
                                        valsT[pp_][hh * 64:(hh + 1) * 64, csl],
                                        raw[:, ssl], 1.0, bc[0:64, ssl],
                                        op0=mybir.AluOpType.mult,
                                        op1=mybir.AluOpType.mult,
                                    )
                                emit_outproj(ii_, range(si, si + 1))
                        attn_base[0] = tc.cur_priority

                if reps == 1:
                    emit_body()
                else:
                    with tc.For_i(0, reps, 1):
                        emit_body()

    _split_excess_waits(nc)
    return nc


_program_cache = None


def _get_program():
    global _program_cache
    if _program_cache is None:
        _program_cache = _build_program()
    return _program_cache


# ---------------------------------------------------------------------------
# Host-side sharding + gather
# ---------------------------------------------------------------------------

def _shard_inputs(x, qkv_w, qkv_b, out_w):
    """Build the 8 per-core input maps."""
    x = np.asarray(x, np.float32)
    qkv_w = np.asarray(qkv_w, np.float32)
    qkv_b = np.asarray(qkv_b, np.float32)
    out_w = np.asarray(out_w, np.float32)

    # per-head q/k/v rows of the fused projection: head h covers rows
    # [h*3*HD, (h+1)*3*HD) split q | k | v
    qw = np.stack([qkv_w[h * 3 * HD: h * 3 * HD + HD] for h in range(NH)])
    kw = np.stack([qkv_w[h * 3 * HD + HD: h * 3 * HD + 2 * HD] for h in range(NH)])
    vw = np.stack([qkv_w[h * 3 * HD + 2 * HD: h * 3 * HD + 3 * HD] for h in range(NH)])
    qb = np.stack([qkv_b[h * 3 * HD: h * 3 * HD + HD] for h in range(NH)])
    kb = np.stack([qkv_b[h * 3 * HD + HD: h * 3 * HD + 2 * HD] for h in range(NH)])
    vb = np.stack([qkv_b[h * 3 * HD + 2 * HD: h * 3 * HD + 3 * HD] for h in range(NH)])

    xT = [np.ascontiguousarray(x[b].T.astype(ml_dtypes.bfloat16)) for b in range(B)]
    ones_in = np.ones((128, 64), np.float32)

    in_maps = []
    for c in range(NCORES):
        b = c // HPC
        g = c % HPC
        hs = slice(g * HPC, (g + 1) * HPC)
        # [4, HD, D] -> [4*HD, D]
        Wq = (SCALE * qw[hs]).reshape(HPC * HD, D)
        Wk = kw[hs].reshape(HPC * HD, D)
        Wv = vw[hs].reshape(HPC * HD, D)
        bq = (SCALE * qb[hs]).reshape(HPC * HD)
        bk = kb[hs].reshape(HPC * HD)
        bv = vb[hs].reshape(HPC * HD)

        wqk_c = np.ascontiguousarray(
            np.concatenate([Wq, Wk], 0).T.astype(ml_dtypes.bfloat16))  # [D, 512]
        bqk_full = np.concatenate([bq, bk])                          # [512]
        bqk_c = np.ascontiguousarray(bqk_full.reshape(-1, 128).T)    # [128, 4]
        wv_c = np.ascontiguousarray(Wv.T.astype(ml_dtypes.bfloat16))  # [D, 256]
        bvr_c = np.ascontiguousarray(np.broadcast_to(bv, (128, HPC * HD)))
        # out_w columns for these heads, transposed: [256, D]
        cols = np.arange(g * HPC * HD, (g + 1) * HPC * HD)
        ot_c = np.ascontiguousarray(out_w[:, cols].T.astype(ml_dtypes.bfloat16))

        in_maps.append({
            "xT": xT[b],
            "wqk": wqk_c,
            "bqk": bqk_c,
            "wv": wv_c,
            "bvr": bvr_c,
            "ot": ot_c,
            "ones_in": ones_in,
        })
    return in_maps


def kernel(x, qkv_w, qkv_b, out_w, out_b):
    nc = _get_program()
    in_maps = _shard_inputs(x, qkv_w, qkv_b, out_w)
    res = run_bass_kernel_spmd(nc, in_maps, core_ids=list(range(NCORES)))
    parts = [res.results[c]["out_p"] for c in range(NCORES)]
    out_b = np.asarray(out_b, np.float32)
    out = np.empty((B, S, D), np.float32)
    for b in range(B):
        acc = np.zeros((S, D), np.float64)
        for g in range(HPC):
            acc += parts[b * HPC + g]
        out[b] = (acc + out_b[None, :]).astype(np.float32)
    return out



# revision 35
# speedup vs baseline: 1.0670x; 1.0670x over previous
"""Multi-head attention (B=2, S=2048, D=1024, H=16, hd=64) on 8 TRN2 cores.

Sharding: tensor-parallel on heads x data-parallel on batch. Core c handles
batch b=c//4 and heads [4*(c%4), 4*(c%4)+4). Each core computes the fused QKV
projection for its head slice, attention for its 4 heads, and a partial
out-projection (out_w column slice); the host sums the 4 partials per batch
and adds out_b.

On-device layout (all matmul inputs fp32r -> full-rate PE):
- qk^T [e, s]: scores^T tiles need dd on partitions; produced directly by
  lhsT=W^T, rhs=x^T. q weights+bias pre-scaled by 1/sqrt(hd) on host.
- v natural [s, dd] (+ ones column per head): pv matmul lhsT=v gives
  vals^T [dd, i] AND the softmax row sums in psum row 64.
- scores^T via K=64 row-packed head pairs (partitions 0-63 / 64-127).
- exp on ACT over [128, 2048] psum regions (amortizes ACT overhead).
- normalize vals^T with reciprocal row sums broadcast via sbuf->sbuf DMA.
"""

import ml_dtypes
import numpy as np

import concourse.bass as bass
import concourse.mybir as mybir
import concourse.tile as tile
from concourse.bass_utils import run_bass_kernel_spmd
from concourse.vector_clock import ScopedClock

B = 2
S = 2048
D = 1024
NH = 16
HD = 64
NCORES = 8
HPC = 4            # heads per core
F32 = mybir.dt.float32
F32R = mybir.dt.float32r
BF16 = mybir.dt.bfloat16

SCALE = 1.0 / np.sqrt(HD)
KT = D // 128      # 8 contraction tiles for the projections
NI = S // 512      # 4 i-chunks (query)
NJ = S // 128      # 16 j-tiles (key)
VW = HD + 2        # 66: v columns + ones column + pad (4B-aligned bf16 slices)


# ---------------------------------------------------------------------------
# Walrus workaround: this toolchain rejects instructions carrying more than
# one sem wait. Split excess waits onto injected same-engine NoOps placed
# directly before the instruction (same-engine program order preserves
# semantics). DMA completion updates are left untouched.
# ---------------------------------------------------------------------------

def _patched_drain_and_barrier(self, tick_clock, wait_clock):
    nc = self.nc
    collector = nc.sync.nop()
    wait_clock.add_sem_waits(
        collector.ins, ScopedClock({None: tick_clock.global_clock})
    )
    si = collector.ins.sync_info
    if si is not None:
        waits = list(si.on_wait or [])
        if len(waits) > 1:
            collector.ins.sync_info = mybir.SyncInfo(
                on_wait=[waits[0]], on_update=list(si.on_update or [])
            )
            for w in waits[1:]:
                n2 = nc.sync.nop()
                n2.ins.sync_info = mybir.SyncInfo(on_wait=[w], on_update=[])
    nc.sync.drain()
    nc.all_engine_barrier()
    popped = nc._tile_sem_poison_stack.pop()
    assert popped is self._sem_poison
    nc.clear_and_free_semaphores(list(self.sems.allocated().values()))
    nc.all_engine_barrier()


def _split_excess_waits(nc, limit=1):
    import bass_rust

    n_split = 0
    for f in nc.m.functions:
        for bb in f.blocks:
            out = []
            for inst in bb.instructions:
                si = inst.sync_info
                waits = list(si.on_wait) if si is not None and si.on_wait else []
                lim = limit
                if len(waits) > lim:
                    for w in waits[:-lim]:
                        nop = bass_rust.InstNoOp(
                            name=f"{inst.name}-waitsplit-{n_split}",
                            ins=[], outs=[], engine=inst.engine,
                        )
                        nop.sync_info = mybir.SyncInfo(on_wait=[w], on_update=[])
                        out.append(nop)
                        n_split += 1
                    inst.sync_info = mybir.SyncInfo(
                        on_wait=waits[-lim:],
                        on_update=list(si.on_update) if si.on_update else [],
                    )
                out.append(inst)
            bb.instructions[:] = out
    return n_split


# ---------------------------------------------------------------------------
# Program build (identical SPMD program on all 8 cores; shards differ in data)
# ---------------------------------------------------------------------------

def _build_program(reps=1, phase="full"):
    """reps>1 repeats the whole computation (idempotently) inside one NEFF —
    used by the benchmark to separate HW exec time from dispatch overhead.
    phase: "proj" | "attn" | "full" — truncated variants for HW bisection."""
    tile.TileContext._drain_and_barrier = _patched_drain_and_barrier

    nc = bass.Bass("TRN2", target_bir_lowering=False, debug=False,
                   num_devices=NCORES)

    xT = nc.dram_tensor("xT", [D, S], BF16, kind="ExternalInput").ap()
    wqk = nc.dram_tensor("wqk", [D, 2 * HPC * HD], BF16, kind="ExternalInput").ap()
    bqk = nc.dram_tensor("bqk", [128, 2 * HPC * HD // 128], F32, kind="ExternalInput").ap()
    wv = nc.dram_tensor("wv", [D, HPC * HD], BF16, kind="ExternalInput").ap()
    bvr = nc.dram_tensor("bvr", [128, HPC * HD], F32, kind="ExternalInput").ap()
    ot = nc.dram_tensor("ot", [2 * 128, D], BF16, kind="ExternalInput").ap()
    ones_in = nc.dram_tensor("ones_in", [128, 64], F32R, kind="ExternalInput").ap()
    out_p = nc.dram_tensor("out_p", [S, D], F32, kind="ExternalOutput").ap()

    NQK = 2 * HPC * HD // 128     # 4 e-tiles of qk^T

    with tile.TileContext(nc) as tc:
        with (
            nc.allow_low_precision(reason="fp32r matmul input rounding"),
            tc.tile_pool(name="weights", bufs=1) as wpool,
            tc.tile_pool(name="acts", bufs=1) as apool,
            tc.tile_pool(name="work", bufs=1) as workpool,
        ):
            # ---------------- loads ----------------
            xT_sb = [wpool.tile([128, S], BF16, tag=f"xT{k}", name=f"xT{k}")
                     for k in range(KT)]
            wqk_sb = [wpool.tile([128, 2 * HPC * HD], BF16, tag=f"wqk{k}",
                                 name=f"wqk{k}") for k in range(KT)]
            wv_sb = [wpool.tile([128, HPC * HD], BF16, tag=f"wv{k}",
                                name=f"wv{k}") for k in range(KT)]
            bqk_sb = wpool.tile([128, NQK], F32, tag="bqk")
            bvr_sb = wpool.tile([128, HPC * HD], F32, tag="bvr")
            ones_sb = wpool.tile([128, 64], F32R, tag="ones")
            ot_sb = [wpool.tile([128, D], BF16, tag=f"ot{k}", name=f"ot{k}")
                     for k in range(2)]
            nc.sync.dma_start(out=bqk_sb[:], in_=bqk[:])
            nc.sync.dma_start(out=bvr_sb[:], in_=bvr[:])
            nc.sync.dma_start(out=ones_sb[:], in_=ones_in[:])
            for k in range(KT):
                nc.sync.dma_start(out=wqk_sb[k][:], in_=wqk[k * 128:(k + 1) * 128, :])
                nc.sync.dma_start(out=xT_sb[k][:], in_=xT[k * 128:(k + 1) * 128, :])
                nc.sync.dma_start(out=wv_sb[k][:], in_=wv[k * 128:(k + 1) * 128, :])
            for k in range(2):
                nc.sync.dma_start(out=ot_sb[k][:], in_=ot[k * 128:(k + 1) * 128, :])

            # persistent activation buffers
            qk_sb = [apool.tile([128, S], BF16, tag=f"qk{t}", name=f"qk{t}")
                     for t in range(NQK)]
            v_sb = [apool.tile([128, HPC * VW], BF16, tag=f"v{j}", name=f"v{j}")
                    for j in range(NJ)]
            valsT = [apool.tile([128, S], BF16, tag=f"vals{p}", name=f"vals{p}")
                     for p in range(2)]
            # constant ones columns of v (softmax row-sum trick), written once
            for j in range(NJ):
                v_dst = v_sb[j][:, 0:HPC * VW].rearrange(
                    "p (h e) -> p h e", h=HPC, e=VW)
                nc.vector.memset(v_dst[:, :, HD:VW], 1.0)

            # ---------------- single psum pool: proj/outproj share tag "pp"
            # (2 banks) + wide (4) + pv (2) = 8 banks, no phase barrier so
            # attention interleaves with late projection tiles.
            with (
                tc.tile_pool(name="ps", bufs=1, space="PSUM") as psp,
                tc.tile_pool(name="attn_sb", bufs=3) as ab,
                tc.tile_pool(name="norm_sb", bufs=2) as nb,
                tc.tile_pool(name="out_sb", bufs=3) as ob,
                tc.tile_pool(name="dram_sc", bufs=4, space="DRAM") as dsc,
            ):
                def emit_qk_tile(t, i, borrow_wide=False):
                    # one qk^T e-tile [e, s-chunk]; bias added on eviction.
                    # Early tiles may borrow the (still idle) "wide" psum
                    # slots so more accumulations overlap the input DMA.
                    if borrow_wide:
                        qs = psp.tile([128, 512], F32, tag="wide", bufs=2,
                                      name="qs")
                    else:
                        qs = psp.tile([128, 512], F32, tag="pp", bufs=2,
                                      name="qs")
                    for k in range(KT):
                        nc.tensor.matmul(
                            qs[:],
                            wqk_sb[k][:, t * 128:(t + 1) * 128],
                            xT_sb[k][:, i * 512:(i + 1) * 512],
                            start=(k == 0), stop=(k == KT - 1),
                        )
                    nc.vector.tensor_scalar_add(
                        qk_sb[t][:, i * 512:(i + 1) * 512], qs[:],
                        bqk_sb[:, t:t + 1],
                    )

                def emit_qk_proj(ts_pair, borrow_wide=False):
                    n_emitted = 0
                    for i in range(NI):
                        for t in ts_pair:
                            emit_qk_tile(t, i, borrow_wide and n_emitted < 2)
                            n_emitted += 1

                def emit_v_tile(j):
                    # v natural [s, dd] + bias (ones columns written once)
                    vs = psp.tile([128, HPC * HD], F32, tag="pp", bufs=2, name="vs")
                    for k in range(KT):
                        nc.tensor.matmul(
                            vs[:],
                            xT_sb[k][:, j * 128:(j + 1) * 128],
                            wv_sb[k][:],
                            start=(k == 0), stop=(k == KT - 1),
                        )
                    v_dst = v_sb[j][:, 0:HPC * VW].rearrange(
                        "p (h e) -> p h e", h=HPC, e=VW)
                    nc.vector.tensor_tensor(
                        v_dst[:, :, 0:HD],
                        vs[:].rearrange("p (h e) -> p h e", h=HPC, e=HD),
                        bvr_sb[:].rearrange("p (h e) -> p h e", h=HPC, e=HD),
                        op=mybir.AluOpType.add,
                    )

                def emit_v_proj():
                    for j in range(NJ):
                        emit_v_tile(j)

                def emit_attn(p, i, at_start=(), steps=None, no_pv=False):
                    # Software-pipelined: scores/exp for j are emitted one
                    # iteration ahead of the pv matmuls for j-1, so the PE
                    # (in-order) issues the next scores pair instead of
                    # blocking on exp(j) before pv(j). `at_start`/`steps`
                    # inject deferred work (normalize / out-proj of the
                    # previous chunk) into this chunk's pipeline.
                    steps = steps or {}
                    isl = slice(i * 512, (i + 1) * 512)
                    for fn in at_start:
                        fn()
                    if no_pv:
                        pvA = pvB = None
                    else:
                        pvA = psp.tile([128, 512], F32, tag="pv", bufs=2, name="pvA")
                        pvB = psp.tile([128, 512], F32, tag="pv", bufs=2, name="pvB")
                    hA, hB = 2 * p, 2 * p + 1
                    es_q = {}
                    for j in range(NJ + 1):
                        if j < NJ:
                            jsl = slice(j * 128, (j + 1) * 128)
                            wide = psp.tile([128, 1024], F32, tag="wide",
                                            bufs=2, name="wide")
                            # scores^T: row-packed pair (head A rows 0:64,
                            # head B rows 64:128 of the PE array)
                            nc.tensor.matmul(
                                wide[:, 0:512],
                                qk_sb[2 + p][0:64, jsl],
                                qk_sb[p][0:64, isl], start=True, stop=True)
                            nc.tensor.matmul(
                                wide[:, 512:1024],
                                qk_sb[2 + p][64:128, jsl],
                                qk_sb[p][64:128, isl], start=True, stop=True)
                            es = ab.tile([128, 1024], BF16, tag="es", bufs=6,
                                         name="es")
                            nc.scalar.activation(
                                es[:], wide[:], mybir.ActivationFunctionType.Exp)
                            es_q[j] = es
                        if j >= 1 and not no_pv:
                            es = es_q.pop(j - 1)
                            nc.tensor.matmul(
                                pvA[0:VW, :],
                                v_sb[j - 1][:, hA * VW:(hA + 1) * VW],
                                es[:, 0:512],
                                start=(j - 1 == 0), stop=(j - 1 == NJ - 1))
                            nc.tensor.matmul(
                                pvB[0:VW, :],
                                v_sb[j - 1][:, hB * VW:(hB + 1) * VW],
                                es[:, 512:1024],
                                start=(j - 1 == 0), stop=(j - 1 == NJ - 1))
                        for fn in steps.get(j, ()):
                            fn()
                    return pvA, pvB

                def norm_evict(pvA, pvB):
                    # evict pv psum banks asap: vals rows to raw tiles; the
                    # two sum rows packed at partitions 0/32 of one tile so a
                    # single DVE reciprocal covers both heads (its cost is
                    # free-size only; partitions run in parallel lanes)
                    raws = []
                    sums = nb.tile([33, 512], F32, tag="sums", bufs=2,
                                   name="sums")
                    for hh, pv in enumerate((pvA, pvB)):
                        raw = nb.tile([HD, 512], F32, tag="raw", bufs=3,
                                      name="raw")
                        nc.vector.tensor_copy(raw[:], pv[0:HD, :])
                        nc.vector.tensor_copy(sums[32 * hh:32 * hh + 1, :],
                                              pv[HD:HD + 1, :])
                        raws.append(raw)
                    return raws, sums

                def norm_recip(sums):
                    # one iterative DVE reciprocal for both heads (strided
                    # partition AP touches only rows 0 and 32); keeps the
                    # norm off the saturated ACT queue mid-body
                    recip = nb.tile([33, 512], F32R, tag="recip", bufs=2,
                                    name="recip")
                    nc.vector.reciprocal(recip[:], sums[:])
                    return recip

                def norm_recip_act(sums):
                    # tail variant: 1/Z = exp(-ln Z) on the (idle-at-tail)
                    # ACT engine; ln/exp share an act table so no reloads
                    recips = []
                    for hh in range(2):
                        lnz = nb.tile([1, 512], F32, tag="lnz", bufs=2,
                                      name="lnz")
                        nc.scalar.activation(
                            lnz[:], sums[32 * hh:32 * hh + 1, :],
                            mybir.ActivationFunctionType.Ln)
                        rc = nb.tile([1, 512], F32R, tag="recip2", bufs=2,
                                     name="recip2")
                        nc.scalar.activation(
                            rc[:], lnz[:],
                            mybir.ActivationFunctionType.Exp, scale=-1.0)
                        recips.append(rc)
                    return recips

                def norm_bcmm(recip):
                    # broadcast recip rows across 64 partitions via a DRAM
                    # round-trip (stride-0 read DMA); entirely off the PE,
                    # and the stt is scheduled steps later so the DMA
                    # latency never blocks the in-order DVE queue
                    bcs = []
                    for hh in range(2):
                        rd = dsc.tile([1, 512], F32R, tag="rd", name="rd")
                        nc.sync.dma_start(out=rd[:],
                                          in_=recip[32 * hh:32 * hh + 1, :])
                        bc = nb.tile([64, 512], F32R, tag="bc", bufs=3,
                                     name="bc")
                        nc.sync.dma_start(out=bc[:],
                                          in_=rd[:].to_broadcast((64, 512)))
                        bcs.append(bc)
                    return bcs

                def norm_stt(p, i, raws, bcs):
                    isl = slice(i * 512, (i + 1) * 512)
                    for hh, (raw, bc) in enumerate(zip(raws, bcs)):
                        nc.vector.scalar_tensor_tensor(
                            valsT[p][hh * 64:(hh + 1) * 64, isl],
                            raw[:], 1.0, bc[:],
                            op0=mybir.AluOpType.mult,
                            op1=mybir.AluOpType.mult,
                        )

                def emit_outproj(i, si_range=range(4)):
                    # out projection for this i-chunk
                    for si in si_range:
                        s0 = i * 512 + si * 128
                        for e in range(2):
                            op = psp.tile([128, 512], F32, tag="pp", bufs=2, name="op")
                            for k in range(2):
                                nc.tensor.matmul(
                                    op[:],
                                    valsT[k][:, s0:s0 + 128],
                                    ot_sb[k][:, e * 512:(e + 1) * 512],
                                    start=(k == 0), stop=(k == 1),
                                )
                            osb = ob.tile([128, 512], F32, tag="osb", name="osb")
                            nc.vector.tensor_copy(osb[:], op[:])
                            nc.sync.dma_start(
                                out=out_p[s0:s0 + 128, e * 512:(e + 1) * 512],
                                in_=osb[:],
                            )

                # Projections are emitted first (dep-tracking needs
                # program order = data order), but attention + out-proj get a
                # low priority band so the scheduler treats projection work
                # as filler once attention tiles become data-ready.
                attn_base = [1]

                def prio():
                    return tc.high_priority(offset=tc.cur_priority - attn_base[0])

                def emit_boot_wave():
                    # k-major first wave over all 8 psum banks: pair-0's
                    # eight qk e-tiles (t 0,2 x i 0..3) accumulate one
                    # k-chunk at a time, so the PE starts as soon as
                    # (wqk[0], xT[0]) lands instead of stalling each tile's
                    # full k-loop on the slowest DMA chunk.
                    pps = [psp.tile([128, 512], F32, tag="pp", bufs=2,
                                    name="bootpp") for _ in range(2)]
                    wides = [psp.tile([128, 1024], F32, tag="wide", bufs=2,
                                      name="bootw") for _ in range(2)]
                    pvs = [psp.tile([128, 512], F32, tag="pv", bufs=2,
                                    name="bootpv") for _ in range(2)]
                    aps = [pps[0][:], pps[1][:],
                           wides[0][:, 0:512], wides[0][:, 512:1024],
                           wides[1][:, 0:512], wides[1][:, 512:1024],
                           pvs[0][:], pvs[1][:]]
                    order = [(0, 0), (2, 0), (0, 1), (2, 1),
                             (0, 2), (2, 2), (0, 3), (2, 3)]
                    for k in range(KT):
                        for s, (t, i) in enumerate(order):
                            nc.tensor.matmul(
                                aps[s],
                                wqk_sb[k][:, t * 128:(t + 1) * 128],
                                xT_sb[k][:, i * 512:(i + 1) * 512],
                                start=(k == 0), stop=(k == KT - 1),
                            )
                    for s, (t, i) in enumerate(order):
                        nc.vector.tensor_scalar_add(
                            qk_sb[t][:, i * 512:(i + 1) * 512], aps[s],
                            bqk_sb[:, t:t + 1],
                        )

                def emit_body():
                    # boot wave gives pair-0 q/k immediately; v tiles follow
                    # (attention chunk (0, 0) consumes them j-progressively),
                    # pair-1 qk tiles last (their attention is much later).
                    emit_boot_wave()
                    for j in range(NJ):
                        emit_v_tile(j)
                    emit_qk_proj((1, 3))
                    if phase == "proj":
                        return
                    if phase == "attn_nopv":
                        for p in range(2):
                            for i in range(NI):
                                with prio():
                                    emit_attn(p, i, no_pv=True)
                                    attn_base[0] = tc.cur_priority
                        return
                    chunks = [(p, i) for p in range(2) for i in range(NI)]
                    pend = None        # previous chunk awaiting normalize
                    pend_out = None    # i-chunk awaiting out-projection
                    for (p, i) in chunks:
                        at_start = []
                        steps = {}
                        if pend is not None:
                            pp_, ii_, pvA_, pvB_ = pend
                            box = []
                            sbox = []
                            rbox = []
                            bcbox = []
                            at_start.append(
                                lambda a=pvA_, b=pvB_, bx=box, sb=sbox:
                                    (lambda rs: (bx.extend(rs[0]),
                                                 sb.append(rs[1])))(
                                        norm_evict(a, b)))
                            if phase in ("full", "noout"):
                                steps.setdefault(1, []).append(
                                    lambda sb=sbox, rb=rbox: rb.append(
                                        norm_recip(sb[0])))
                                steps.setdefault(5, []).append(
                                    lambda rb=rbox, bb=bcbox: bb.extend(
                                        norm_bcmm(rb[0])))
                                steps.setdefault(8, []).append(
                                    lambda p2=pp_, i2=ii_, bx=box, bb=bcbox:
                                        norm_stt(p2, i2, bx, bb))
                            if pp_ == 1 and phase == "full":
                                # out-projection for ii_ right after its stt
                                for k, si in enumerate((10, 12, 14, 15)):
                                    steps.setdefault(si, []).append(
                                        lambda i2=ii_, k2=k: emit_outproj(
                                            i2, range(k2, k2 + 1)))
                                pend_out = None
                        with prio():
                            pvA, pvB = emit_attn(p, i, at_start, steps)
                            attn_base[0] = tc.cur_priority
                        pend = (p, i, pvA, pvB)
                    # tail: last chunk's normalize + final out-projections,
                    # pipelined at si (128-query) granularity so the first
                    # out-proj matmuls start ~3 stt-slices earlier
                    with prio():
                        pp_, ii_, pvA_, pvB_ = pend
                        raws, sums = norm_evict(pvA_, pvB_)
                        if phase in ("full", "noout"):
                            # broadcast via K=1 matmul — the PE is idle in
                            # the tail and psum banks are free, so this
                            # replaces the ~4.5us DMA round-trip latency
                            # with a ~0.25us matmul
                            bcs = []
                            for recip in norm_recip_act(sums):
                                bcps = psp.tile([128, 512], F32, tag="pv",
                                                bufs=2, name="bcps")
                                nc.tensor.matmul(
                                    bcps[0:64, :], ones_sb[0:1, 0:64],
                                    recip[:], start=True, stop=True)
                                bcs.append(bcps)
                            for si in range(4):
                                ssl = slice(si * 128, (si + 1) * 128)
                                csl = slice(ii_ * 512 + si * 128,
                                            ii_ * 512 + (si + 1) * 128)
                                for hh, (raw, bc) in enumerate(zip(raws, bcs)):
                                    nc.vector.scalar_tensor_tensor(
                                        valsT[pp_][hh * 64:(hh + 1) * 64, csl],
                                        raw[:, ssl], 1.0, bc[0:64, ssl],
                                        op0=mybir.AluOpType.mult,
                                        op1=mybir.AluOpType.mult,
                                    )
                                emit_outproj(ii_, range(si, si + 1))
                        attn_base[0] = tc.cur_priority

                if reps == 1:
                    emit_body()
                else:
                    with tc.For_i(0, reps, 1):
                        emit_body()

    _split_excess_waits(nc)
    return nc


_program_cache = None


def _get_program():
    global _program_cache
    if _program_cache is None:
        _program_cache = _build_program()
    return _program_cache


# ---------------------------------------------------------------------------
# Host-side sharding + gather
# ---------------------------------------------------------------------------

def _shard_inputs(x, qkv_w, qkv_b, out_w):
    """Build the 8 per-core input maps."""
    x = np.asarray(x, np.float32)
    qkv_w = np.asarray(qkv_w, np.float32)
    qkv_b = np.asarray(qkv_b, np.float32)
    out_w = np.asarray(out_w, np.float32)

    # per-head q/k/v rows of the fused projection: head h covers rows
    # [h*3*HD, (h+1)*3*HD) split q | k | v
    qw = np.stack([qkv_w[h * 3 * HD: h * 3 * HD + HD] for h in range(NH)])
    kw = np.stack([qkv_w[h * 3 * HD + HD: h * 3 * HD + 2 * HD] for h in range(NH)])
    vw = np.stack([qkv_w[h * 3 * HD + 2 * HD: h * 3 * HD + 3 * HD] for h in range(NH)])
    qb = np.stack([qkv_b[h * 3 * HD: h * 3 * HD + HD] for h in range(NH)])
    kb = np.stack([qkv_b[h * 3 * HD + HD: h * 3 * HD + 2 * HD] for h in range(NH)])
    vb = np.stack([qkv_b[h * 3 * HD + 2 * HD: h * 3 * HD + 3 * HD] for h in range(NH)])

    xT = [np.ascontiguousarray(x[b].T.astype(ml_dtypes.bfloat16)) for b in range(B)]
    ones_in = np.ones((128, 64), np.float32)

    in_maps = []
    for c in range(NCORES):
        b = c // HPC
        g = c % HPC
        hs = slice(g * HPC, (g + 1) * HPC)
        # [4, HD, D] -> [4*HD, D]
        Wq = (SCALE * qw[hs]).reshape(HPC * HD, D)
        Wk = kw[hs].reshape(HPC * HD, D)
        Wv = vw[hs].reshape(HPC * HD, D)
        bq = (SCALE * qb[hs]).reshape(HPC * HD)
        bk = kb[hs].reshape(HPC * HD)
        bv = vb[hs].reshape(HPC * HD)

        wqk_c = np.ascontiguousarray(
            np.concatenate([Wq, Wk], 0).T.astype(ml_dtypes.bfloat16))  # [D, 512]
        bqk_full = np.concatenate([bq, bk])                          # [512]
        bqk_c = np.ascontiguousarray(bqk_full.reshape(-1, 128).T)    # [128, 4]
        wv_c = np.ascontiguousarray(Wv.T.astype(ml_dtypes.bfloat16))  # [D, 256]
        bvr_c = np.ascontiguousarray(np.broadcast_to(bv, (128, HPC * HD)))
        # out_w columns for these heads, transposed: [256, D]
        cols = np.arange(g * HPC * HD, (g + 1) * HPC * HD)
        ot_c = np.ascontiguousarray(out_w[:, cols].T.astype(ml_dtypes.bfloat16))

        in_maps.append({
            "xT": xT[b],
            "wqk": wqk_c,
            "bqk": bqk_c,
            "wv": wv_c,
            "bvr": bvr_c,
            "ot": ot_c,
            "ones_in": ones_in,
        })
    return in_maps


def kernel(x, qkv_w, qkv_b, out_w, out_b):
    nc = _get_program()
    in_maps = _shard_inputs(x, qkv_w, qkv_b, out_w)
    res = run_bass_kernel_spmd(nc, in_maps, core_ids=list(range(NCORES)))
    parts = [res.results[c]["out_p"] for c in range(NCORES)]
    out_b = np.asarray(out_b, np.float32)
    out = np.empty((B, S, D), np.float32)
    for b in range(B):
        acc = np.zeros((S, D), np.float64)
        for g in range(HPC):
            acc += parts[b * HPC + g]
        out[b] = (acc + out_b[None, :]).astype(np.float32)
    return out

